# revision 7
# baseline (speedup 1.0000x reference)
"""Trainium2 Bass kernel for nn_CrossSlideConsistencyLoss.

Computes, for 3 slides of 8192 2-D points each:
  - radial histogram (20 bins) of centered radii
  - |FFT|[0:5] of the mean-centered angular histogram (72 bins)
  - collision rate: fraction of points whose nearest neighbor is < 0.01 away
then the mean over descriptor components of the across-slide variance (ddof=1).

Strategy (8 NeuronCores, SPMD):
  - Host sorts each slide's points by x; any pair closer than 0.01 is within
    W ranks (validated at runtime), so the NxN cdist collapses to a banded
    window per 128-row block: one K=5 matmul per block then one fused
    compare+accumulate pass (ACT sign+accum for half the blocks, DVE
    is_gt+accum for the rest). 1024 rows/core, 24 blocks/core.
  - Angles via octant-reduced arctan: q = dy/dx or dx/dy (always in [-1,1]),
    one Act.Arctan pass, branchless quadrant fixup; |FFT| bins 1..4 equal
    direct sums of cos/sin(2 pi k aidx / 72), summed by the final AllReduce.
  - Radial histogram: one core per slide; the 20-threshold compare runs on
    the (otherwise idle) Pool engine, partition reduction on PE via
    ones-matmuls, so the tail never waits on it.
  - One 352B AllReduce carries [radial fractions (diffed+masked) | collision
    rates | cos/sin pairs]; the post-collective variance is 6 ops.
"""
import numpy as np

import concourse.bass as bass
import concourse.bacc as bacc
import concourse.bass_isa as bass_isa
import concourse.mybir as mybir
import concourse.tile as tile
from concourse.bass_utils import run_bass_kernel_spmd

F32 = mybir.dt.float32
F32R = mybir.dt.float32r
Alu = mybir.AluOpType
Act = mybir.ActivationFunctionType

N = 8192
N_CORES = 8
NSLIDES = 3
ROWS_PER_CORE = N // N_CORES          # 1024
NBLK = ROWS_PER_CORE // 128           # 8 blocks per core per slide
SH = NSLIDES * NBLK                   # 24 shard columns
NACT_S = 3                            # collision blocks per slide on ACT
TH = 1e-4                             # d^2 threshold (0.01^2)
PI = float(np.pi)
R2C = 12582912.0                      # 1.5 * 2^23: rne magic constant

# AllReduce vector layout ([1, 88] f32), component-major (c, s):
#   [0:60)   radial hist fractions (c=0..19, s minor), diffed+masked+1/N
#   [60:63)  collision rates per slide (c=20)
#   [63:87)  DFT sums as (cos, sin) pairs, i=(k-1)*3+s major (c=21..24)
V_COLL = 60
V_DFT = 63
V_LEN = 128


def _bcast(ap, axis_len, at):
    """Insert a broadcast (stride-0) dim of length axis_len at free position
    `at` (0 = before the flattened free dim, 1 = after it)."""
    p, f = ap.shape[0], int(np.prod(ap.shape[1:]))
    if at == 0:
        return ap.rearrange("p (a b) -> p a b", a=1).to_broadcast([p, axis_len, f])
    return ap.rearrange("p (a b) -> p a b", b=1).to_broadcast([p, f, axis_len])


def build_program(win, collective=True):
    buf = ROWS_PER_CORE + win - 128   # rhs window buffer length per core/slide
    bufp = buf // 128                 # p-major free dim of the buffer
    assert buf % 128 == 0

    # mega-input column layout
    # m128 [128, F1]: pxy(3*128) | rxy(128) | pw(3*2*bufp) | xsh(24) | ysh(24)
    #                 | kvec(20) | k4(4) | mask row0 (64)
    O_PXY, O_RXY = 0, 384
    O_PW = O_RXY + 128
    O_XSH = O_PW + NSLIDES * 2 * bufp
    O_YSH = O_XSH + SH
    O_KVEC = O_YSH + SH
    O_K4 = O_KVEC + 20
    O_MASK = O_K4 + 4
    F1 = O_MASK + 64
    # m5 [5, F2]: rhs buffers (3*buf; row4 device-filled) | lhs (3*1024)
    O_LHS = NSLIDES * buf
    F2 = O_LHS + NSLIDES * ROWS_PER_CORE

    nc = bacc.Bacc("TRN2", target_bir_lowering=False, debug=False, num_devices=N_CORES)
    i_m128 = nc.dram_tensor("m128", [128, F1], F32, kind="ExternalInput")
    i_m5 = nc.dram_tensor("m5", [5, F2], F32R, kind="ExternalInput")
    o_out = nc.dram_tensor("out", [1, 1], F32, kind="ExternalOutput")

    with tile.TileContext(nc) as tc:
        with (
            tc.tile_pool(name="cst", bufs=1) as cst,
            tc.tile_pool(name="scr", bufs=3) as scp,
            tc.tile_pool(name="psum", bufs=4, space="PSUM") as pp,
            tc.tile_pool(name="psv", bufs=1, space="PSUM") as pv,
            tc.tile_pool(name="dram", bufs=1, space="DRAM") as dr,
        ):
            # ---------------- input loads (pxy+rxy first: centers dep) -------
            big128 = cst.tile([128, F1], F32, tag="big128")
            nc.sync.dma_start(big128[:, 0:512], i_m128[:, 0:512])
            nc.sync.dma_start(big128[:, 512:F1], i_m128[:, 512:F1])
            big5 = cst.tile([5, F2], F32R, tag="big5")
            nc.sync.dma_start(big5[:], i_m5[:])

            rxy = big128[:, O_RXY:O_RXY + 128]
            pws = [big128[:, O_PW + 2 * bufp * s:O_PW + 2 * bufp * (s + 1)] for s in range(NSLIDES)]
            xysh = big128[:, O_XSH:O_XSH + 2 * SH]
            kvecT = big128[:, O_KVEC:O_KVEC + 20]
            k4T = big128[:, O_K4:O_K4 + 4]
            mask60 = big128[0:1, O_MASK:O_MASK + 60]
            rhs = [big5[:, buf * s:buf * (s + 1)] for s in range(NSLIDES)]
            lhsT = big5[:, O_LHS:O_LHS + NSLIDES * ROWS_PER_CORE]

            ones128 = cst.tile([128, 1], F32, tag="ones128")
            nc.gpsimd.memset(ones128[:], 1.0)
            invN128 = cst.tile([128, 1], F32, tag="invN128")
            nc.gpsimd.memset(invN128[:], 1.0 / N)
            nc.scalar.add_instruction(mybir.InstLoadActFuncSet(
                act_func_set_id=9, name=f"I-{nc.next_id()}", ins=[], outs=[]))

            # ---------------- centers: one reduce + one matmul ----------------
            cs8 = scp.tile([128, 8], F32, tag="cs8")
            nc.vector.tensor_reduce(
                cs8[:].rearrange("p (g t) -> p g t", g=4),
                big128[:, 0:512].rearrange("p (g t f) -> p g t f", g=4, t=2),
                mybir.AxisListType.X, Alu.add,
            )
            c8p = pv.tile([1, 8], F32, tag="c8p")
            nc.tensor.matmul(c8p[:], ones128[:], cs8[:])

            # crowXY [1,48] layout (t, s, b): x-centers then y-centers
            crowXY = cst.tile([1, 48], F32, tag="crowXY")
            nc.scalar.activation(
                crowXY[0:1, 0:24].rearrange("p (s b) -> p s b", s=NSLIDES),
                c8p[0:1, 0:6].rearrange("p (s t) -> p s t", s=NSLIDES)[:, :, 0:1]
                .to_broadcast([1, NSLIDES, NBLK]),
                Act.Copy, scale=1.0 / N,
            )
            nc.scalar.activation(
                crowXY[0:1, 24:48].rearrange("p (s b) -> p s b", s=NSLIDES),
                c8p[0:1, 0:6].rearrange("p (s t) -> p s t", s=NSLIDES)[:, :, 1:2]
                .to_broadcast([1, NSLIDES, NBLK]),
                Act.Copy, scale=1.0 / N,
            )
            c2t = cst.tile([1, 2], F32, tag="c2t")
            nc.scalar.activation(c2t[:], c8p[0:1, 6:8], Act.Copy, scale=1.0 / N)
            cXY24 = cst.tile([128, 48], F32, tag="cXY24")
            nc.gpsimd.partition_broadcast(cXY24[:], crowXY[:])
            crB = cst.tile([128, 2], F32, tag="crB")
            nc.gpsimd.partition_broadcast(crB[:], c2t[:])

            # ---------------- rhs row 4: sq_j over the window buffer ----------
            for s in range(NSLIDES):
                xw = pws[s][:, 0:bufp]
                yw = pws[s][:, bufp:2 * bufp]
                sqw = cst.tile([128, bufp], F32, tag=f"sqw{s}")
                nc.gpsimd.tensor_tensor(sqw[:], xw, xw, Alu.mult)
                sqw2 = cst.tile([128, bufp], F32, tag=f"sqw2{s}")
                nc.gpsimd.tensor_tensor(sqw2[:], yw, yw, Alu.mult)
                nc.gpsimd.tensor_tensor(sqw[:], sqw[:], sqw2[:], Alu.add)
                sqwr = cst.tile([128, bufp], F32R, tag=f"sqwr{s}")
                nc.vector.tensor_copy(sqwr[:], sqw[:])
                nc.sync.dma_start(
                    rhs[s][4:5, :].rearrange("o (p f) -> o p f", p=128), sqwr[:]
                )

            # ------- shard prep on gpsimd (collision thresholds) -------
            squ = cst.tile([128, 2 * SH], F32, tag="squ")
            nc.gpsimd.tensor_tensor(squ[:], xysh, xysh, Alu.mult)
            sqsh = cst.tile([128, SH], F32, tag="sqsh")
            nc.gpsimd.tensor_tensor(sqsh[:], squ[:, 0:SH], squ[:, SH:2 * SH], Alu.add)
            biasA = cst.tile([128, SH], F32, tag="biasA")
            nc.gpsimd.tensor_scalar(biasA[:], sqsh[:], -1.0, TH, Alu.mult, Alu.add)
            thrD = cst.tile([128, SH], F32, tag="thrD")
            nc.gpsimd.tensor_scalar(thrD[:], sqsh[:], TH, None, Alu.subtract)

            # ---------------- angle chain: octant-reduced arctan --------------
            dxy = scp.tile([128, 48], F32, tag="dxy")
            nc.vector.tensor_tensor(dxy[:], xysh, cXY24[:], Alu.subtract)
            dx, dy = dxy[:, 0:SH], dxy[:, SH:2 * SH]
            # quadrant terms on Pool
            neg = scp.tile([128, SH], F32, tag="neg")
            nc.gpsimd.tensor_scalar(neg[:], dx, 0.0, None, Alu.is_lt)
            sy36 = scp.tile([128, SH], F32, tag="sy36")
            nc.gpsimd.tensor_scalar(sy36[:], dy, 0.0, 72.0, Alu.is_ge, Alu.mult)
            nc.gpsimd.tensor_scalar(sy36[:], sy36[:], 36.0, None, Alu.subtract)
            pn36p = scp.tile([128, SH], F32, tag="pn36p")
            nc.gpsimd.tensor_tensor(pn36p[:], neg[:], sy36[:], Alu.mult)
            nc.gpsimd.tensor_scalar(pn36p[:], pn36p[:], 36.0, None, Alu.add)
            s18v = scp.tile([128, SH], F32, tag="s18v")
            nc.gpsimd.tensor_scalar(s18v[:], sy36[:], 0.5, 36.0, Alu.mult, Alu.add)
            # swap mask on Pool: |dy| > |dx|
            sq48 = scp.tile([128, 48], F32, tag="sq48")
            nc.gpsimd.tensor_tensor(sq48[:], dxy[:], dxy[:], Alu.mult)
            swap = scp.tile([128, SH], F32, tag="swap")
            nc.gpsimd.tensor_tensor(swap[:], sq48[:, SH:2 * SH], sq48[:, 0:SH],
                                    Alu.subtract)
            nc.gpsimd.tensor_scalar(swap[:], swap[:], 0.0, None, Alu.is_gt)
            # crossed reciprocals: invc = [1/dy | 1/dx]
            invc = scp.tile([128, 48], F32, tag="invc")
            nc.vector.reciprocal(invc[:, 0:SH], dy)
            nc.vector.reciprocal(invc[:, SH:2 * SH], dx)
            q21 = scp.tile([128, 48], F32, tag="q21")
            nc.vector.tensor_tensor(q21[:], dxy[:], invc[:], Alu.mult)
            nc.vector.tensor_scalar(q21[:], q21[:], -1.0009, 1.0009, Alu.max, Alu.min)
            at21 = scp.tile([128, 48], F32, tag="at21")
            at_bi = nc.scalar.activation(at21[:], q21[:], Act.Arctan)
            # u = angle in [0, 72) turns*72; u1 = main branch, u2 = swapped
            ut = scp.tile([128, SH], F32, tag="ut")
            nc.vector.scalar_tensor_tensor(ut[:], at21[:, SH:2 * SH], 36.0 / PI,
                                           pn36p[:], Alu.mult, Alu.add)
            u2s = scp.tile([128, SH], F32, tag="u2s")
            nc.vector.scalar_tensor_tensor(u2s[:], at21[:, 0:SH], -36.0 / PI,
                                           s18v[:], Alu.mult, Alu.add)
            # branchless blend: u = u1 + swap*(u2 - u1)
            du = scp.tile([128, SH], F32, tag="du")
            nc.vector.tensor_tensor(du[:], u2s[:], ut[:], Alu.subtract)
            nc.vector.tensor_tensor(du[:], du[:], swap[:], Alu.mult)
            nc.vector.tensor_tensor(ut[:], ut[:], du[:], Alu.add)
            # negative floor via rne trick: floor = rne - (rne > u); nfl = -floor
            rv = scp.tile([128, SH], F32, tag="rv")
            nc.vector.tensor_scalar(rv[:], ut[:], R2C, R2C, Alu.add, Alu.subtract)
            cmp = scp.tile([128, SH], F32, tag="cmp")
            nc.vector.tensor_tensor(cmp[:], rv[:], ut[:], Alu.is_gt)
            nfl = scp.tile([128, SH], F32, tag="nfl")
            nc.vector.tensor_tensor(nfl[:], cmp[:], rv[:], Alu.subtract)

            # ---------------- radial: owned slide, Pool compare + PE reduce ---
            dfxy = scp.tile([128, 128], F32, tag="dfxy")
            nc.gpsimd.tensor_scalar(dfxy[:, 0:64], rxy[:, 0:64], crB[:, 0:1],
                                    None, Alu.subtract)
            nc.gpsimd.tensor_scalar(dfxy[:, 64:128], rxy[:, 64:128], crB[:, 1:2],
                                    None, Alu.subtract)
            sqf = scp.tile([128, 128], F32, tag="sqf")
            nc.vector.tensor_tensor(sqf[:], dfxy[:], dfxy[:], Alu.mult)
            rf2 = scp.tile([128, 64], F32, tag="rf2")
            nc.vector.tensor_tensor(rf2[:], sqf[:, 0:64], sqf[:, 64:128], Alu.add)
            rmx = scp.tile([128, 1], F32, tag="rmx")
            nc.vector.tensor_reduce(rmx[:], rf2[:], mybir.AxisListType.X, Alu.max)
            rmxB = scp.tile([128, 1], F32, tag="rmxB")
            nc.gpsimd.partition_all_reduce(rmxB[:], rmx[:], 128, bass_isa.ReduceOp.max)
            rm1 = scp.tile([128, 1], F32, tag="rm1")
            nc.vector.tensor_scalar(rm1[:], rmxB[:], 1e-8, None, Alu.add)
            thrT = scp.tile([128, 20], F32, tag="thrT")
            nc.vector.tensor_scalar(thrT[:], kvecT, rm1[:, 0:1], None, Alu.mult)
            # 20-threshold compare + reduce on DVE, partition contraction on PE
            ct = cst.tile([128, 1280], F32, tag="ct")
            nc.vector.tensor_tensor(
                ct[:].rearrange("p (k f) -> p k f", k=20),
                _bcast(rf2[:], 20, 0), _bcast(thrT[:], 64, 1), Alu.is_lt,
            )
            cr = scp.tile([128, 20], F32, tag="cr")
            nc.vector.tensor_reduce(
                cr[:], ct[:].rearrange("p (k f) -> p k f", k=20),
                mybir.AxisListType.X, Alu.add,
            )
            cP = pv.tile([1, 20], F32, tag="cP")
            nc.tensor.matmul(cP[:], ones128[:], cr[:])
            c20s = scp.tile([1, 20], F32, tag="c20s")
            nc.vector.tensor_copy(c20s[:], cP[:])
            d20 = scp.tile([1, 20], F32, tag="d20")
            nc.vector.tensor_copy(d20[0:1, 0:1], c20s[0:1, 0:1])
            nc.vector.tensor_tensor(d20[0:1, 1:20], c20s[0:1, 1:20],
                                    c20s[0:1, 0:19], Alu.subtract)

            # ---------------- collision blocks (ACT + DVE split) --------------
            acc = cst.tile([128, SH], F32, tag="acc")
            sign_insts = []
            for s in range(NSLIDES):
                for b in range(NBLK):
                    col = s * NBLK + b
                    zp = pp.tile([128, win], F32, tag="zp")
                    for off in range(0, win, 512):  # moving free dim cap is 512
                        nc.tensor.matmul(
                            zp[:, off:off + min(512, win - off)],
                            lhsT[:, s * ROWS_PER_CORE + b * 128:
                                 s * ROWS_PER_CORE + b * 128 + 128],
                            rhs[s][:, b * 128 + off:b * 128 + off + min(512, win - off)],
                        )
                    if b < NACT_S:
                        sg = scp.tile([128, win], F32, tag="sg")
                        sign_insts.append(nc.scalar.activation(
                            sg[:], zp[:], Act.Sign,
                            bias=biasA[:, col:col + 1], accum_out=acc[:, col:col + 1],
                        ).ins)
                    else:
                        sg = scp.tile([128, win], F32, tag="sgd")
                        nc.vector.tensor_scalar(
                            sg[:], zp[:], thrD[:, col:col + 1], None, Alu.is_gt,
                            Alu.add, accum_out=acc[:, col:col + 1],
                        )
            # ACT cols: sumsign > 3-win <=> count >= 2; DVE cols: count > 1.5
            ind = scp.tile([128, SH], F32, tag="ind")
            for s in range(NSLIDES):
                o = s * NBLK
                nc.vector.tensor_scalar(ind[:, o:o + NACT_S], acc[:, o:o + NACT_S],
                                        float(3.0 - win), None, Alu.is_gt)
                nc.vector.tensor_scalar(ind[:, o + NACT_S:o + NBLK],
                                        acc[:, o + NACT_S:o + NBLK],
                                        1.5, None, Alu.is_gt)
            indR = scp.tile([128, NSLIDES], F32, tag="indR")
            nc.vector.tensor_reduce(
                indR[:], ind[:].rearrange("p (s b) -> p s b", s=NSLIDES),
                mybir.AxisListType.X, Alu.add,
            )
            vecpC = pv.tile([1, 3], F32, tag="vecpC")
            nc.tensor.matmul(vecpC[:], invN128[:], indR[:])

            # ---------------- trig: one fused [cos | sin] pass ----------------
            # vb lower = (k/72)*(-aidx) (sin args), upper = +0.25 turns (cos)
            vb = scp.tile([128, 8 * SH], F32, tag="vb")
            nc.vector.scalar_tensor_tensor(
                vb[:, 0:4 * SH].rearrange("p (k f) -> p k f", k=4),
                _bcast(nfl[:], 4, 0), -71.0, _bcast(k4T, SH, 1), Alu.max, Alu.mult,
            )
            nc.vector.tensor_scalar(vb[:, 4 * SH:8 * SH], vb[:, 0:4 * SH], 0.25,
                                    None, Alu.add)
            mb = scp.tile([128, 8 * SH], F32, tag="mb")
            nc.gpsimd.tensor_scalar(mb[:], vb[:], R2C, R2C, Alu.add, Alu.subtract)
            nc.gpsimd.tensor_tensor(mb[:], vb[:], mb[:], Alu.subtract)
            scv = scp.tile([128, 8 * SH], F32, tag="scv")
            sin_bi = nc.scalar.activation(scv[:], mb[:], Act.Sin, scale=2.0 * PI)
            if len(sign_insts) > 7:
                bass._add_dep_helper(sin_bi.ins, sign_insts[7], False,
                                     "order: trig sin after 8 collision signs")
            # block layout: cos (k,s) in [0:12], sin in [12:24]
            pairs24 = cst.tile([128, 24], F32, tag="pairs24")
            nc.vector.tensor_reduce(
                pairs24[:, 0:12].rearrange("p (k s) -> p k s", k=4),
                scv[:, 4 * SH:8 * SH].rearrange("p (k s b) -> p k s b", k=4, s=NSLIDES),
                mybir.AxisListType.X, Alu.add,
            )
            nc.vector.tensor_reduce(
                pairs24[:, 12:24].rearrange("p (k s) -> p k s", k=4),
                scv[:, 0:4 * SH].rearrange("p (k s b) -> p k s b", k=4, s=NSLIDES),
                mybir.AxisListType.X, Alu.add,
            )
            vecp24 = pv.tile([1, 24], F32, tag="vecp24")
            nc.tensor.matmul(vecp24[:], ones128[:], pairs24[:])

            # ---------------- assemble + AllReduce ----------------
            vecS = cst.tile([1, V_LEN], F32, tag="vecS")
            nc.gpsimd.memset(vecS[:], 0.0)
            nc.vector.tensor_tensor(
                vecS[0:1, 0:60].rearrange("p (c s) -> p c s", c=20),
                _bcast(d20[:], NSLIDES, 1), mask60.rearrange("p (c s) -> p c s", c=20),
                Alu.mult,
            )
            nc.vector.tensor_copy(vecS[0:1, V_COLL:V_COLL + 3], vecpC[:])
            nc.vector.tensor_copy(vecS[0:1, V_DFT:V_DFT + 24], vecp24[:])

            # preload the sqrt act-table before the collective so the post
            # stage pays no table load in the tail
            s2t = scp.tile([1, 1], F32, tag="s2t")
            nc.vector.tensor_tensor(s2t[:], pairs24[0:1, 0:1], pairs24[0:1, 0:1],
                                    Alu.mult)
            dum = scp.tile([1, 1], F32, tag="dum")
            dum_bi = nc.scalar.activation(dum[:], s2t[:], Act.Sqrt)
            if sign_insts:
                bass._add_dep_helper(dum_bi.ins, sign_insts[-1], False,
                                     "order: sqrt table preload after collision signs")

            ccin = dr.tile([1, V_LEN], F32)
            ccout = dr.tile([1, V_LEN], F32, addr_space="Shared")
            nc.sync.dma_start(ccin[:], vecS[:])
            if collective:
                nc.gpsimd.collective_compute(
                    "AllReduce", Alu.add,
                    replica_groups=[list(range(N_CORES))],
                    ins=[ccin.opt()], outs=[ccout.opt()],
                )
            else:
                nc.sync.dma_start(ccout[:], ccin[:])
            vecR = cst.tile([1, V_LEN], F32, tag="vecR")
            nc.sync.dma_start(vecR[:], ccout[:])

            # ---------------- descriptors + variance ----------------
            # power spectrum k=1..4: sqrt(cos^2 + sin^2), in place over pairs
            t24 = scp.tile([1, 24], F32, tag="t24")
            nc.vector.tensor_tensor(t24[:], vecR[0:1, V_DFT:V_DFT + 24],
                                    vecR[0:1, V_DFT:V_DFT + 24], Alu.mult)
            ps2 = scp.tile([1, 12], F32, tag="ps2")
            nc.vector.tensor_tensor(ps2[:], t24[0:1, 0:12], t24[0:1, 12:24], Alu.add)
            nc.scalar.activation(vecR[0:1, V_DFT:V_DFT + 12], ps2[:], Act.Sqrt)
            # variance over slides (ddof=1), mean over 26 components
            SCv = vecR[0:1, 0:75].rearrange("p (c s) -> p c s", c=25)
            m25 = scp.tile([1, 25], F32, tag="m25")
            nc.vector.tensor_reduce(m25[:], SCv, mybir.AxisListType.X, Alu.add)
            dev = scp.tile([1, 75], F32, tag="dev")
            nc.vector.scalar_tensor_tensor(
                dev[:].rearrange("p (c s) -> p c s", c=25),
                _bcast(m25[:], NSLIDES, 1), 1.0 / NSLIDES, SCv,
                Alu.mult, Alu.subtract,
            )
            nc.vector.tensor_tensor(dev[:], dev[:], dev[:], Alu.mult)
            tot = scp.tile([1, 1], F32, tag="tot")
            nc.vector.tensor_reduce(
                tot[:], dev[:].rearrange("p (a f) -> p a f", a=1),
                mybir.AxisListType.XY, Alu.add,
            )
            nc.vector.tensor_scalar(tot[:], tot[:], 1.0 / (2.0 * 26.0), None,
                                    Alu.mult)
            nc.sync.dma_start(o_out[:], tot[:])

    nc.compile()
    return nc


_PROG_CACHE = {}


def _get_program(win):
    if win not in _PROG_CACHE:
        _PROG_CACHE[win] = build_program(win)
    return _PROG_CACHE[win]


def _host_prep(coords_list, win):
    whalf = (win - 128) // 2
    buf = ROWS_PER_CORE + win - 128
    bufp = buf // 128
    SENT_X = np.float32(1e6)

    O_PXY, O_RXY = 0, 384
    O_PW = O_RXY + 128
    O_XSH = O_PW + NSLIDES * 2 * bufp
    O_YSH = O_XSH + SH
    O_KVEC = O_YSH + SH
    O_K4 = O_KVEC + 20
    O_MASK = O_K4 + 4
    F1 = O_MASK + 64
    O_LHS = NSLIDES * buf
    F2 = O_LHS + NSLIDES * ROWS_PER_CORE

    sxy = []
    for c in coords_list:
        order = np.argsort(c[:, 0], kind="stable")
        sxy.append(np.ascontiguousarray(c[order]))

    base128 = np.zeros((128, F1), np.float32)
    for s in range(NSLIDES):
        base128[:, O_PXY + 128 * s:O_PXY + 128 * s + 64] = sxy[s][:, 0].reshape(128, 64)
        base128[:, O_PXY + 128 * s + 64:O_PXY + 128 * (s + 1)] = sxy[s][:, 1].reshape(128, 64)
    base128[:, O_KVEC:O_KVEC + 20] = (np.arange(1, 21, dtype=np.float32) / np.float32(20)) ** 2
    base128[:, O_K4:O_K4 + 4] = np.arange(1, 5, dtype=np.float32) / np.float32(72)

    in_maps = []
    for core in range(N_CORES):
        r0 = core * ROWS_PER_CORE
        m128 = base128.copy()
        m5 = np.zeros((5, F2), np.float32)
        for s in range(NSLIDES):
            xs, ys = sxy[s][:, 0], sxy[s][:, 1]
            sl = slice(O_LHS + s * ROWS_PER_CORE, O_LHS + (s + 1) * ROWS_PER_CORE)
            m5[0, sl] = xs[r0:r0 + ROWS_PER_CORE]
            m5[1, sl] = ys[r0:r0 + ROWS_PER_CORE]
            m5[2, sl] = m5[0, sl]
            m5[3, sl] = m5[1, sl]
            m5[4, sl] = -1.0
            # window buffer [r0-whalf, r0+1024+whalf) with sentinel padding
            xb = np.full(buf, SENT_X, np.float32)
            yb = np.zeros(buf, np.float32)
            g0 = r0 - whalf
            lo, hi = max(g0, 0), min(g0 + buf, N)
            xb[lo - g0:hi - g0] = xs[lo:hi]
            yb[lo - g0:hi - g0] = ys[lo:hi]
            m5[0, s * buf:(s + 1) * buf] = xb
            m5[1, s * buf:(s + 1) * buf] = yb
            m5[2, s * buf:(s + 1) * buf] = xb
            m5[3, s * buf:(s + 1) * buf] = yb
            m128[:, O_PW + 2 * bufp * s:O_PW + 2 * bufp * s + bufp] = xb.reshape(128, bufp)
            m128[:, O_PW + 2 * bufp * s + bufp:O_PW + 2 * bufp * (s + 1)] = yb.reshape(128, bufp)
            for b in range(NBLK):
                m128[:, O_XSH + s * NBLK + b] = xs[r0 + b * 128:r0 + b * 128 + 128]
                m128[:, O_YSH + s * NBLK + b] = ys[r0 + b * 128:r0 + b * 128 + 128]
        # radial: owned slide (cores 0-2), mask row 0 (c-major, s minor)
        m128[:, O_RXY:O_RXY + 64] = sxy[core % NSLIDES][:, 0].reshape(128, 64)
        m128[:, O_RXY + 64:O_RXY + 128] = sxy[core % NSLIDES][:, 1].reshape(128, 64)
        if core < NSLIDES:
            m128[0, O_MASK + np.arange(20) * 3 + core] = np.float32(1.0) / np.float32(N)
        in_maps.append({"m128": m128, "m5": m5})
    return in_maps


def _pick_win(coords_list):
    # win > 2048 would need a deeper PSUM chunking scheme; these whalf values
    # cover any remotely Gaussian-like input (the shipped inputs pass at 64)
    for whalf in (64, 192, 448, 960):
        ok = True
        for c in coords_list:
            xs = np.sort(c[:, 0])
            if (xs[whalf:] - xs[:-whalf]).min() < 0.01:
                ok = False
                break
        if ok:
            return 128 + 2 * whalf
    raise ValueError("no valid rank window (pathological input)")


def kernel(coords0, coords1, coords2, slide_labels=None, **_):
    coords_list = [np.ascontiguousarray(np.asarray(c, dtype=np.float32))
                   for c in (coords0, coords1, coords2)]
    assert coords_list[0].shape == (N, 2)
    win = _pick_win(coords_list)
    nc = _get_program(win)
    in_maps = _host_prep(coords_list, win)
    res = run_bass_kernel_spmd(nc, in_maps, core_ids=list(range(N_CORES)))
    val = np.float32(res.results[0]["out"][0, 0])
    return np.asarray(val, dtype=np.float32).reshape(())



# revision 10
# speedup vs baseline: 1.0825x; 1.0825x over previous
"""Trainium2 Bass kernel for nn_CrossSlideConsistencyLoss.

Computes, for 3 slides of 8192 2-D points each:
  - radial histogram (20 bins) of centered radii
  - |FFT|[0:5] of the mean-centered angular histogram (72 bins)
  - collision rate: fraction of points whose nearest neighbor is < 0.01 away
then the mean over descriptor components of the across-slide variance (ddof=1).

Strategy (8 NeuronCores, SPMD):
  - Host sorts each slide's points by x; any pair closer than 0.01 is within
    W ranks (validated at runtime), so the NxN cdist collapses to a banded
    window per 128-row block: one K=5 matmul per block then one fused
    compare+accumulate pass (ACT sign+accum for half the blocks, DVE
    is_gt+accum for the rest). 1024 rows/core, 24 blocks/core.
  - Angles via octant-reduced arctan: q = dy/dx or dx/dy (always in [-1,1]),
    one Act.Arctan pass, branchless quadrant fixup; |FFT| bins 1..4 equal
    direct sums of cos/sin(2 pi k aidx / 72), summed by the final AllReduce.
  - Radial histogram: one core per slide; the 20-threshold compare runs on
    the (otherwise idle) Pool engine, partition reduction on PE via
    ones-matmuls, so the tail never waits on it.
  - One 352B AllReduce carries [radial fractions (diffed+masked) | collision
    rates | cos/sin pairs]; the post-collective variance is 6 ops.
"""
import numpy as np

import concourse.bass as bass
import concourse.bacc as bacc
import concourse.bass_isa as bass_isa
import concourse.mybir as mybir
import concourse.tile as tile
from concourse.bass_utils import run_bass_kernel_spmd

F32 = mybir.dt.float32
F32R = mybir.dt.float32r
Alu = mybir.AluOpType
Act = mybir.ActivationFunctionType

N = 8192
N_CORES = 8
NSLIDES = 3
ROWS_PER_CORE = N // N_CORES          # 1024
NBLK = ROWS_PER_CORE // 128           # 8 blocks per core per slide
SH = NSLIDES * NBLK                   # 24 shard columns
NACT_S = 3                            # collision blocks per slide on ACT
TH = 1e-4                             # d^2 threshold (0.01^2)
PI = float(np.pi)
R2C = 12582912.0                      # 1.5 * 2^23: rne magic constant

# AllReduce vector layout ([1, 88] f32), component-major (c, s):
#   [0:60)   radial hist fractions (c=0..19, s minor), diffed+masked+1/N
#   [60:63)  collision rates per slide (c=20)
#   [63:87)  DFT sums as (cos, sin) pairs, i=(k-1)*3+s major (c=21..24)
V_COLL = 60
V_DFT = 63
V_LEN = 128


def _bcast(ap, axis_len, at):
    """Insert a broadcast (stride-0) dim of length axis_len at free position
    `at` (0 = before the flattened free dim, 1 = after it)."""
    p, f = ap.shape[0], int(np.prod(ap.shape[1:]))
    if at == 0:
        return ap.rearrange("p (a b) -> p a b", a=1).to_broadcast([p, axis_len, f])
    return ap.rearrange("p (a b) -> p a b", b=1).to_broadcast([p, f, axis_len])


def build_program(win, collective=True):
    buf = ROWS_PER_CORE + win - 128   # rhs window buffer length per core/slide
    bufp = buf // 128                 # p-major free dim of the buffer
    assert buf % 128 == 0

    # mega-input column layout
    # m128 [128, F1]: pxy(3*128) | rxy(128) | pw(3*2*bufp) | xsh(24) | ysh(24)
    #                 | kvec(20) | k4(4) | mask row0 (64)
    O_PXY, O_RXY = 0, 384
    O_PW = O_RXY + 128
    O_XSH = O_PW + NSLIDES * 2 * bufp
    O_YSH = O_XSH + SH
    O_KVEC = O_YSH + SH
    O_K4 = O_KVEC + 20
    O_MASK = O_K4 + 4
    F1 = O_MASK + 64
    # m5 [5, F2]: rhs buffers (3*buf; row4 device-filled) | lhs (3*1024)
    O_LHS = NSLIDES * buf
    F2 = O_LHS + NSLIDES * ROWS_PER_CORE

    nc = bacc.Bacc("TRN2", target_bir_lowering=False, debug=False, num_devices=N_CORES)
    i_m128 = nc.dram_tensor("m128", [128, F1], F32, kind="ExternalInput")
    i_m5 = nc.dram_tensor("m5", [5, F2], F32R, kind="ExternalInput")
    o_out = nc.dram_tensor("out", [1, 1], F32, kind="ExternalOutput")

    with tile.TileContext(nc) as tc:
        with (
            tc.tile_pool(name="cst", bufs=1) as cst,
            tc.tile_pool(name="scr", bufs=3) as scp,
            tc.tile_pool(name="psum", bufs=4, space="PSUM") as pp,
            tc.tile_pool(name="psv", bufs=1, space="PSUM") as pv,
            tc.tile_pool(name="dram", bufs=1, space="DRAM") as dr,
        ):
            # ---------------- input loads (pxy+rxy first: centers dep) -------
            big128 = cst.tile([128, F1], F32, tag="big128")
            nc.sync.dma_start(big128[:, 0:512], i_m128[:, 0:512])
            nc.sync.dma_start(big128[:, 512:F1], i_m128[:, 512:F1])
            big5 = cst.tile([5, F2], F32R, tag="big5")
            nc.sync.dma_start(big5[:], i_m5[:])

            rxy = big128[:, O_RXY:O_RXY + 128]
            pws = [big128[:, O_PW + 2 * bufp * s:O_PW + 2 * bufp * (s + 1)] for s in range(NSLIDES)]
            xysh = big128[:, O_XSH:O_XSH + 2 * SH]
            kvecT = big128[:, O_KVEC:O_KVEC + 20]
            k4T = big128[:, O_K4:O_K4 + 4]
            mask60 = big128[0:1, O_MASK:O_MASK + 60]
            rhs = [big5[:, buf * s:buf * (s + 1)] for s in range(NSLIDES)]
            lhsT = big5[:, O_LHS:O_LHS + NSLIDES * ROWS_PER_CORE]

            ones128 = cst.tile([128, 1], F32, tag="ones128")
            nc.gpsimd.memset(ones128[:], 1.0)
            invN128 = cst.tile([128, 1], F32, tag="invN128")
            nc.gpsimd.memset(invN128[:], 1.0 / N)
            nc.scalar.add_instruction(mybir.InstLoadActFuncSet(
                act_func_set_id=9, name=f"I-{nc.next_id()}", ins=[], outs=[]))

            # ---------------- centers: one reduce + one matmul ----------------
            cs8 = scp.tile([128, 8], F32, tag="cs8")
            nc.vector.tensor_reduce(
                cs8[:].rearrange("p (g t) -> p g t", g=4),
                big128[:, 0:512].rearrange("p (g t f) -> p g t f", g=4, t=2),
                mybir.AxisListType.X, Alu.add,
            )
            c8p = pv.tile([1, 8], F32, tag="c8p")
            nc.tensor.matmul(c8p[:], ones128[:], cs8[:])

            # crowXY [1,48] layout (t, s, b): x-centers then y-centers
            crowXY = cst.tile([1, 48], F32, tag="crowXY")
            nc.scalar.activation(
                crowXY[0:1, 0:24].rearrange("p (s b) -> p s b", s=NSLIDES),
                c8p[0:1, 0:6].rearrange("p (s t) -> p s t", s=NSLIDES)[:, :, 0:1]
                .to_broadcast([1, NSLIDES, NBLK]),
                Act.Copy, scale=1.0 / N,
            )
            nc.scalar.activation(
                crowXY[0:1, 24:48].rearrange("p (s b) -> p s b", s=NSLIDES),
                c8p[0:1, 0:6].rearrange("p (s t) -> p s t", s=NSLIDES)[:, :, 1:2]
                .to_broadcast([1, NSLIDES, NBLK]),
                Act.Copy, scale=1.0 / N,
            )
            c2t = cst.tile([1, 2], F32, tag="c2t")
            nc.scalar.activation(c2t[:], c8p[0:1, 6:8], Act.Copy, scale=1.0 / N)
            cXY24 = cst.tile([128, 48], F32, tag="cXY24")
            nc.gpsimd.partition_broadcast(cXY24[:], crowXY[:])
            crB = cst.tile([128, 2], F32, tag="crB")
            nc.gpsimd.partition_broadcast(crB[:], c2t[:])

            # ---------------- rhs row 4: sq_j over the window buffer ----------
            for s in range(NSLIDES):
                xw = pws[s][:, 0:bufp]
                yw = pws[s][:, bufp:2 * bufp]
                sqw = cst.tile([128, bufp], F32, tag=f"sqw{s}")
                nc.gpsimd.tensor_tensor(sqw[:], xw, xw, Alu.mult)
                sqw2 = cst.tile([128, bufp], F32, tag=f"sqw2{s}")
                nc.gpsimd.tensor_tensor(sqw2[:], yw, yw, Alu.mult)
                nc.gpsimd.tensor_tensor(sqw[:], sqw[:], sqw2[:], Alu.add)
                sqwr = cst.tile([128, bufp], F32R, tag=f"sqwr{s}")
                nc.vector.tensor_copy(sqwr[:], sqw[:])
                nc.sync.dma_start(
                    rhs[s][4:5, :].rearrange("o (p f) -> o p f", p=128), sqwr[:]
                )

            # ------- shard prep on gpsimd (collision thresholds) -------
            squ = cst.tile([128, 2 * SH], F32, tag="squ")
            nc.gpsimd.tensor_tensor(squ[:], xysh, xysh, Alu.mult)
            sqsh = cst.tile([128, SH], F32, tag="sqsh")
            nc.gpsimd.tensor_tensor(sqsh[:], squ[:, 0:SH], squ[:, SH:2 * SH], Alu.add)
            biasA = cst.tile([128, SH], F32, tag="biasA")
            nc.gpsimd.tensor_scalar(biasA[:], sqsh[:], -1.0, TH, Alu.mult, Alu.add)
            thrD = cst.tile([128, SH], F32, tag="thrD")
            nc.gpsimd.tensor_scalar(thrD[:], sqsh[:], TH, None, Alu.subtract)

            # ---------------- angle chain: octant-reduced arctan --------------
            dxy = scp.tile([128, 48], F32, tag="dxy")
            nc.vector.tensor_tensor(dxy[:], xysh, cXY24[:], Alu.subtract)
            dx, dy = dxy[:, 0:SH], dxy[:, SH:2 * SH]
            # quadrant terms on Pool
            neg = scp.tile([128, SH], F32, tag="neg")
            nc.gpsimd.tensor_scalar(neg[:], dx, 0.0, None, Alu.is_lt)
            sy36 = scp.tile([128, SH], F32, tag="sy36")
            nc.gpsimd.tensor_scalar(sy36[:], dy, 0.0, 72.0, Alu.is_ge, Alu.mult)
            nc.gpsimd.tensor_scalar(sy36[:], sy36[:], 36.0, None, Alu.subtract)
            pn36p = scp.tile([128, SH], F32, tag="pn36p")
            nc.gpsimd.tensor_tensor(pn36p[:], neg[:], sy36[:], Alu.mult)
            nc.gpsimd.tensor_scalar(pn36p[:], pn36p[:], 36.0, None, Alu.add)
            s18v = scp.tile([128, SH], F32, tag="s18v")
            nc.gpsimd.tensor_scalar(s18v[:], sy36[:], 0.5, 36.0, Alu.mult, Alu.add)
            # swap mask on Pool: |dy| > |dx|
            sq48 = scp.tile([128, 48], F32, tag="sq48")
            nc.gpsimd.tensor_tensor(sq48[:], dxy[:], dxy[:], Alu.mult)
            swap = scp.tile([128, SH], F32, tag="swap")
            nc.gpsimd.tensor_tensor(swap[:], sq48[:, SH:2 * SH], sq48[:, 0:SH],
                                    Alu.subtract)
            nc.gpsimd.tensor_scalar(swap[:], swap[:], 0.0, None, Alu.is_gt)
            # crossed reciprocals: invc = [1/dy | 1/dx]
            invc = scp.tile([128, 48], F32, tag="invc")
            nc.vector.reciprocal(invc[:, 0:SH], dy)
            nc.vector.reciprocal(invc[:, SH:2 * SH], dx)
            q21 = scp.tile([128, 48], F32, tag="q21")
            nc.vector.tensor_tensor(q21[:], dxy[:], invc[:], Alu.mult)
            nc.vector.tensor_scalar(q21[:], q21[:], -1.0009, 1.0009, Alu.max, Alu.min)
            at21 = scp.tile([128, 48], F32, tag="at21")
            at_bi = nc.scalar.activation(at21[:], q21[:], Act.Arctan)
            # u = angle in [0, 72) turns*72; u1 = main branch, u2 = swapped
            ut = scp.tile([128, SH], F32, tag="ut")
            nc.vector.scalar_tensor_tensor(ut[:], at21[:, SH:2 * SH], 36.0 / PI,
                                           pn36p[:], Alu.mult, Alu.add)
            u2s = scp.tile([128, SH], F32, tag="u2s")
            nc.vector.scalar_tensor_tensor(u2s[:], at21[:, 0:SH], -36.0 / PI,
                                           s18v[:], Alu.mult, Alu.add)
            # branchless blend: u = u1 + swap*(u2 - u1)
            du = scp.tile([128, SH], F32, tag="du")
            nc.vector.tensor_tensor(du[:], u2s[:], ut[:], Alu.subtract)
            nc.vector.tensor_tensor(du[:], du[:], swap[:], Alu.mult)
            nc.vector.tensor_tensor(ut[:], ut[:], du[:], Alu.add)
            # negative floor via rne trick: floor = rne - (rne > u); nfl = -floor
            rv = scp.tile([128, SH], F32, tag="rv")
            nc.vector.tensor_scalar(rv[:], ut[:], R2C, R2C, Alu.add, Alu.subtract)
            cmp = scp.tile([128, SH], F32, tag="cmp")
            nc.vector.tensor_tensor(cmp[:], rv[:], ut[:], Alu.is_gt)
            nfl = scp.tile([128, SH], F32, tag="nfl")
            nc.vector.tensor_tensor(nfl[:], cmp[:], rv[:], Alu.subtract)

            # ---------------- radial: owned slide, Pool compare + PE reduce ---
            dfxy = scp.tile([128, 128], F32, tag="dfxy")
            nc.gpsimd.tensor_scalar(dfxy[:, 0:64], rxy[:, 0:64], crB[:, 0:1],
                                    None, Alu.subtract)
            nc.gpsimd.tensor_scalar(dfxy[:, 64:128], rxy[:, 64:128], crB[:, 1:2],
                                    None, Alu.subtract)
            sqf = scp.tile([128, 128], F32, tag="sqf")
            nc.vector.tensor_tensor(sqf[:], dfxy[:], dfxy[:], Alu.mult)
            rf2 = scp.tile([128, 64], F32, tag="rf2")
            nc.vector.tensor_tensor(rf2[:], sqf[:, 0:64], sqf[:, 64:128], Alu.add)
            rmx = scp.tile([128, 1], F32, tag="rmx")
            nc.vector.tensor_reduce(rmx[:], rf2[:], mybir.AxisListType.X, Alu.max)
            rmxB = scp.tile([128, 1], F32, tag="rmxB")
            nc.gpsimd.partition_all_reduce(rmxB[:], rmx[:], 128, bass_isa.ReduceOp.max)
            rm1 = scp.tile([128, 1], F32, tag="rm1")
            nc.vector.tensor_scalar(rm1[:], rmxB[:], 1e-8, None, Alu.add)
            thrT = scp.tile([128, 20], F32, tag="thrT")
            nc.vector.tensor_scalar(thrT[:], kvecT, rm1[:, 0:1], None, Alu.mult)
            # 20-threshold compare + reduce on DVE, partition contraction on PE
            ct = cst.tile([128, 1280], F32, tag="ct")
            ct_bi = nc.vector.tensor_tensor(
                ct[:].rearrange("p (k f) -> p k f", k=20),
                _bcast(rf2[:], 20, 0), _bcast(thrT[:], 64, 1), Alu.is_lt,
            )
            cr = scp.tile([128, 20], F32, tag="cr")
            nc.vector.tensor_reduce(
                cr[:], ct[:].rearrange("p (k f) -> p k f", k=20),
                mybir.AxisListType.X, Alu.add,
            )
            cP = pv.tile([1, 20], F32, tag="cP")
            cP_bi = nc.tensor.matmul(cP[:], ones128[:], cr[:])
            c20s = scp.tile([1, 20], F32, tag="c20s")
            nc.vector.tensor_copy(c20s[:], cP[:])
            d20 = scp.tile([1, 20], F32, tag="d20")
            nc.vector.tensor_copy(d20[0:1, 0:1], c20s[0:1, 0:1])
            nc.vector.tensor_tensor(d20[0:1, 1:20], c20s[0:1, 1:20],
                                    c20s[0:1, 0:19], Alu.subtract)

            # ---------------- collision blocks (ACT + DVE split) --------------
            acc = cst.tile([128, SH], F32, tag="acc")
            sign_insts = []
            mm_insts = []
            dve_cmp_insts = []
            for s in range(NSLIDES):
                for b in range(NBLK):
                    col = s * NBLK + b
                    zp = pp.tile([128, win], F32, tag="zp")
                    for off in range(0, win, 512):  # moving free dim cap is 512
                        mm_insts.append(nc.tensor.matmul(
                            zp[:, off:off + min(512, win - off)],
                            lhsT[:, s * ROWS_PER_CORE + b * 128:
                                 s * ROWS_PER_CORE + b * 128 + 128],
                            rhs[s][:, b * 128 + off:b * 128 + off + min(512, win - off)],
                        ))
                    if b < NACT_S:
                        sg = scp.tile([128, win], F32, tag="sg")
                        sign_insts.append(nc.scalar.activation(
                            sg[:], zp[:], Act.Sign,
                            bias=biasA[:, col:col + 1], accum_out=acc[:, col:col + 1],
                        ).ins)
                    else:
                        sg = scp.tile([128, win], F32, tag="sgd")
                        dve_cmp_insts.append(nc.vector.tensor_scalar(
                            sg[:], zp[:], thrD[:, col:col + 1], None, Alu.is_gt,
                            Alu.add, accum_out=acc[:, col:col + 1],
                        ))
            if mm_insts:
                bass._add_dep_helper(cP_bi.ins, mm_insts[-1].ins, False,
                                     "order: radial cP after collision mms")
            if len(dve_cmp_insts) > 3:
                bass._add_dep_helper(ct_bi.ins, dve_cmp_insts[3].ins, False,
                                     "order: radial ct after dve cmp 4")
            # ACT cols: sumsign > 3-win <=> count >= 2; DVE cols: count > 1.5
            ind = scp.tile([128, SH], F32, tag="ind")
            for s in range(NSLIDES):
                o = s * NBLK
                nc.vector.tensor_scalar(ind[:, o:o + NACT_S], acc[:, o:o + NACT_S],
                                        float(3.0 - win), None, Alu.is_gt)
                nc.vector.tensor_scalar(ind[:, o + NACT_S:o + NBLK],
                                        acc[:, o + NACT_S:o + NBLK],
                                        1.5, None, Alu.is_gt)
            indR = scp.tile([128, NSLIDES], F32, tag="indR")
            nc.vector.tensor_reduce(
                indR[:], ind[:].rearrange("p (s b) -> p s b", s=NSLIDES),
                mybir.AxisListType.X, Alu.add,
            )
            vecpC = pv.tile([1, 3], F32, tag="vecpC")
            nc.tensor.matmul(vecpC[:], invN128[:], indR[:])

            # ---------------- trig: one fused [cos | sin] pass ----------------
            # vb lower = (k/72)*(-aidx) (sin args), upper = +0.25 turns (cos)
            vb = scp.tile([128, 8 * SH], F32, tag="vb")
            nc.vector.scalar_tensor_tensor(
                vb[:, 0:4 * SH].rearrange("p (k f) -> p k f", k=4),
                _bcast(nfl[:], 4, 0), -71.0, _bcast(k4T, SH, 1), Alu.max, Alu.mult,
            )
            nc.vector.tensor_scalar(vb[:, 4 * SH:8 * SH], vb[:, 0:4 * SH], 0.25,
                                    None, Alu.add)
            mb = scp.tile([128, 8 * SH], F32, tag="mb")
            nc.gpsimd.tensor_scalar(mb[:], vb[:], R2C, R2C, Alu.add, Alu.subtract)
            nc.gpsimd.tensor_tensor(mb[:], vb[:], mb[:], Alu.subtract)
            scv = scp.tile([128, 8 * SH], F32, tag="scv")
            sin_bi = nc.scalar.activation(scv[:], mb[:], Act.Sin, scale=2.0 * PI)
            if len(sign_insts) > 7:
                bass._add_dep_helper(sin_bi.ins, sign_insts[7], False,
                                     "order: trig sin after 8 collision signs")
            # block layout: cos (k,s) in [0:12], sin in [12:24]
            pairs24 = cst.tile([128, 24], F32, tag="pairs24")
            nc.vector.tensor_reduce(
                pairs24[:, 0:12].rearrange("p (k s) -> p k s", k=4),
                scv[:, 4 * SH:8 * SH].rearrange("p (k s b) -> p k s b", k=4, s=NSLIDES),
                mybir.AxisListType.X, Alu.add,
            )
            nc.vector.tensor_reduce(
                pairs24[:, 12:24].rearrange("p (k s) -> p k s", k=4),
                scv[:, 0:4 * SH].rearrange("p (k s b) -> p k s b", k=4, s=NSLIDES),
                mybir.AxisListType.X, Alu.add,
            )
            vecp24 = pv.tile([1, 24], F32, tag="vecp24")
            nc.tensor.matmul(vecp24[:], ones128[:], pairs24[:])

            # ---------------- assemble + AllReduce ----------------
            vecS = cst.tile([1, V_LEN], F32, tag="vecS")
            nc.gpsimd.memset(vecS[:], 0.0)
            nc.vector.tensor_tensor(
                vecS[0:1, 0:60].rearrange("p (c s) -> p c s", c=20),
                _bcast(d20[:], NSLIDES, 1), mask60.rearrange("p (c s) -> p c s", c=20),
                Alu.mult,
            )
            nc.vector.tensor_copy(vecS[0:1, V_COLL:V_COLL + 3], vecpC[:])
            nc.vector.tensor_copy(vecS[0:1, V_DFT:V_DFT + 24], vecp24[:])

            # preload the sqrt act-table before the collective so the post
            # stage pays no table load in the tail
            s2t = scp.tile([1, 1], F32, tag="s2t")
            nc.vector.tensor_tensor(s2t[:], pairs24[0:1, 0:1], pairs24[0:1, 0:1],
                                    Alu.mult)
            dum = scp.tile([1, 1], F32, tag="dum")
            dum_bi = nc.scalar.activation(dum[:], s2t[:], Act.Sqrt)
            if sign_insts:
                bass._add_dep_helper(dum_bi.ins, sign_insts[-1], False,
                                     "order: sqrt table preload after collision signs")

            ccin = dr.tile([1, V_LEN], F32)
            ccout = dr.tile([1, V_LEN], F32, addr_space="Shared")
            nc.sync.dma_start(ccin[:], vecS[:])
            if collective:
                nc.gpsimd.collective_compute(
                    "AllReduce", Alu.add,
                    replica_groups=[list(range(N_CORES))],
                    ins=[ccin.opt()], outs=[ccout.opt()],
                )
            else:
                nc.sync.dma_start(ccout[:], ccin[:])
            vecR = cst.tile([1, V_LEN], F32, tag="vecR")
            nc.sync.dma_start(vecR[:], ccout[:])

            # ---------------- descriptors + variance ----------------
            # power spectrum k=1..4: sqrt(cos^2 + sin^2), in place over pairs
            t24 = scp.tile([1, 24], F32, tag="t24")
            nc.vector.tensor_tensor(t24[:], vecR[0:1, V_DFT:V_DFT + 24],
                                    vecR[0:1, V_DFT:V_DFT + 24], Alu.mult)
            ps2 = scp.tile([1, 12], F32, tag="ps2")
            nc.vector.tensor_tensor(ps2[:], t24[0:1, 0:12], t24[0:1, 12:24], Alu.add)
            nc.scalar.activation(vecR[0:1, V_DFT:V_DFT + 12], ps2[:], Act.Sqrt)
            # variance over slides (ddof=1), mean over 26 components
            SCv = vecR[0:1, 0:75].rearrange("p (c s) -> p c s", c=25)
            m25 = scp.tile([1, 25], F32, tag="m25")
            nc.vector.tensor_reduce(m25[:], SCv, mybir.AxisListType.X, Alu.add)
            dev = scp.tile([1, 75], F32, tag="dev")
            nc.vector.scalar_tensor_tensor(
                dev[:].rearrange("p (c s) -> p c s", c=25),
                _bcast(m25[:], NSLIDES, 1), 1.0 / NSLIDES, SCv,
                Alu.mult, Alu.subtract,
            )
            nc.vector.tensor_tensor(dev[:], dev[:], dev[:], Alu.mult)
            tot = scp.tile([1, 1], F32, tag="tot")
            nc.vector.tensor_reduce(
                tot[:], dev[:].rearrange("p (a f) -> p a f", a=1),
                mybir.AxisListType.XY, Alu.add,
            )
            nc.vector.tensor_scalar(tot[:], tot[:], 1.0 / (2.0 * 26.0), None,
                                    Alu.mult)
            nc.sync.dma_start(o_out[:], tot[:])

    nc.compile()
    return nc


_PROG_CACHE = {}


def _get_program(win):
    if win not in _PROG_CACHE:
        _PROG_CACHE[win] = build_program(win)
    return _PROG_CACHE[win]


def _host_prep(coords_list, win):
    whalf = (win - 128) // 2
    buf = ROWS_PER_CORE + win - 128
    bufp = buf // 128
    SENT_X = np.float32(1e6)

    O_PXY, O_RXY = 0, 384
    O_PW = O_RXY + 128
    O_XSH = O_PW + NSLIDES * 2 * bufp
    O_YSH = O_XSH + SH
    O_KVEC = O_YSH + SH
    O_K4 = O_KVEC + 20
    O_MASK = O_K4 + 4
    F1 = O_MASK + 64
    O_LHS = NSLIDES * buf
    F2 = O_LHS + NSLIDES * ROWS_PER_CORE

    sxy = []
    for c in coords_list:
        order = np.argsort(c[:, 0], kind="stable")
        sxy.append(np.ascontiguousarray(c[order]))

    base128 = np.zeros((128, F1), np.float32)
    for s in range(NSLIDES):
        base128[:, O_PXY + 128 * s:O_PXY + 128 * s + 64] = sxy[s][:, 0].reshape(128, 64)
        base128[:, O_PXY + 128 * s + 64:O_PXY + 128 * (s + 1)] = sxy[s][:, 1].reshape(128, 64)
    base128[:, O_KVEC:O_KVEC + 20] = (np.arange(1, 21, dtype=np.float32) / np.float32(20)) ** 2
    base128[:, O_K4:O_K4 + 4] = np.arange(1, 5, dtype=np.float32) / np.float32(72)

    in_maps = []
    for core in range(N_CORES):
        r0 = core * ROWS_PER_CORE
        m128 = base128.copy()
        m5 = np.zeros((5, F2), np.float32)
        for s in range(NSLIDES):
            xs, ys = sxy[s][:, 0], sxy[s][:, 1]
            sl = slice(O_LHS + s * ROWS_PER_CORE, O_LHS + (s + 1) * ROWS_PER_CORE)
            m5[0, sl] = xs[r0:r0 + ROWS_PER_CORE]
            m5[1, sl] = ys[r0:r0 + ROWS_PER_CORE]
            m5[2, sl] = m5[0, sl]
            m5[3, sl] = m5[1, sl]
            m5[4, sl] = -1.0
            # window buffer [r0-whalf, r0+1024+whalf) with sentinel padding
            xb = np.full(buf, SENT_X, np.float32)
            yb = np.zeros(buf, np.float32)
            g0 = r0 - whalf
            lo, hi = max(g0, 0), min(g0 + buf, N)
            xb[lo - g0:hi - g0] = xs[lo:hi]
            yb[lo - g0:hi - g0] = ys[lo:hi]
            m5[0, s * buf:(s + 1) * buf] = xb
            m5[1, s * buf:(s + 1) * buf] = yb
            m5[2, s * buf:(s + 1) * buf] = xb
            m5[3, s * buf:(s + 1) * buf] = yb
            m128[:, O_PW + 2 * bufp * s:O_PW + 2 * bufp * s + bufp] = xb.reshape(128, bufp)
            m128[:, O_PW + 2 * bufp * s + bufp:O_PW + 2 * bufp * (s + 1)] = yb.reshape(128, bufp)
            for b in range(NBLK):
                m128[:, O_XSH + s * NBLK + b] = xs[r0 + b * 128:r0 + b * 128 + 128]
                m128[:, O_YSH + s * NBLK + b] = ys[r0 + b * 128:r0 + b * 128 + 128]
        # radial: owned slide (cores 0-2), mask row 0 (c-major, s minor)
        m128[:, O_RXY:O_RXY + 64] = sxy[core % NSLIDES][:, 0].reshape(128, 64)
        m128[:, O_RXY + 64:O_RXY + 128] = sxy[core % NSLIDES][:, 1].reshape(128, 64)
        if core < NSLIDES:
            m128[0, O_MASK + np.arange(20) * 3 + core] = np.float32(1.0) / np.float32(N)
        in_maps.append({"m128": m128, "m5": m5})
    return in_maps


def _pick_win(coords_list):
    # win > 2048 would need a deeper PSUM chunking scheme; these whalf values
    # cover any remotely Gaussian-like input (the shipped inputs pass at 64)
    for whalf in (64, 192, 448, 960):
        ok = True
        for c in coords_list:
            xs = np.sort(c[:, 0])
            if (xs[whalf:] - xs[:-whalf]).min() < 0.01:
                ok = False
                break
        if ok:
            return 128 + 2 * whalf
    raise ValueError("no valid rank window (pathological input)")


def kernel(coords0, coords1, coords2, slide_labels=None, **_):
    coords_list = [np.ascontiguousarray(np.asarray(c, dtype=np.float32))
                   for c in (coords0, coords1, coords2)]
    assert coords_list[0].shape == (N, 2)
    win = _pick_win(coords_list)
    nc = _get_program(win)
    in_maps = _host_prep(coords_list, win)
    res = run_bass_kernel_spmd(nc, in_maps, core_ids=list(range(N_CORES)))
    val = np.float32(res.results[0]["out"][0, 0])
    return np.asarray(val, dtype=np.float32).reshape(())

